# revision 1
# baseline (speedup 1.0000x reference)
"""GridNetBlock (TF-GridNet) Trainium2 kernel: 8-core SPMD, 5 launches."""
import sys, os, contextlib
for _p in ("/opt/trn_rl_repo", "/root/.axon_site/_ro/trn_rl_repo"):
    if os.path.isdir(_p) and _p not in sys.path:
        sys.path.insert(0, _p)
import numpy as np
import concourse.bass as bass
import concourse.bacc as bacc
import concourse.tile as tile
from concourse import mybir
from concourse.masks import make_identity
from concourse.bass_utils import run_bass_kernel_spmd

F32 = mybir.dt.float32
BF16 = mybir.dt.bfloat16
AF = mybir.ActivationFunctionType
OP = mybir.AluOpType
AX = mybir.AxisListType

B, C, T, Q = 2, 64, 1000, 65
KS = 4
Qp, L1, Hh, HID, L2 = 68, 17, 128, 256, 250
NH, E, Dv = 4, 4, 16
EPS = 1e-5
NCORES = 8
TSH = T // 4
NP1 = TSH * Qp
G1 = (NP1 + 127) // 128   # 133
RW2 = (B * Qp) // NCORES  # 17
TP = 1003                 # causal padded time
NT1 = L1 * TSH            # 4250
NT2 = L2 * RW2            # 4250


def bap(t, tail):
    ap = list(t.ap)
    for n in tail:
        ap.append([0, n])
    return bass.AP(tensor=t.tensor, offset=t.offset, ap=ap)


def shift_ap(t, off, dims):
    return bass.AP(tensor=t.tensor, offset=t.offset + off, ap=[t.ap[0]] + dims)


def new_nc():
    return bacc.Bacc("TRN2", target_bir_lowering=False, debug=False,
                     enable_asserts=True, num_devices=NCORES)


def ln_posmajor(nc, pool, work, xpm, G, nred, eps_t):
    s1 = work.tile([128, G], F32, tag="lns1")
    nc.vector.tensor_reduce(out=s1[:], in_=xpm[:], axis=AX.X, op=OP.add)
    xsq = pool.tile([128, G, nred], BF16, tag="xut")
    nc.scalar.activation(out=xsq[:], in_=xpm[:], func=AF.Square)
    s2 = work.tile([128, G], F32, tag="lns2")
    nc.vector.tensor_reduce(out=s2[:], in_=xsq[:], axis=AX.X, op=OP.add)
    mu = work.tile([128, G], F32, tag="lnmu")
    nc.vector.tensor_scalar_mul(out=mu[:], in0=s1[:], scalar1=1.0 / nred)
    var = work.tile([128, G], F32, tag="lnvar")
    nc.vector.tensor_tensor(out=var[:], in0=mu[:], in1=mu[:], op=OP.mult)
    nc.vector.scalar_tensor_tensor(out=var[:], in0=s2[:], scalar=1.0 / nred,
                                   in1=var[:], op0=OP.mult, op1=OP.subtract)
    rs = work.tile([128, G], F32, tag="lnrs")
    nc.scalar.activation(out=rs[:], in_=var[:], func=AF.Sqrt, bias=eps_t[:])
    nc.vector.reciprocal(out=rs[:], in_=rs[:])
    zpm = pool.tile([128, G, nred], BF16, tag="xut")
    nc.vector.tensor_tensor(out=zpm[:], in0=xpm[:], in1=bap(mu, [nred]),
                            op=OP.subtract)
    nc.vector.tensor_tensor(out=zpm[:], in0=zpm[:], in1=bap(rs, [nred]),
                            op=OP.mult)
    return zpm


def lstm(nc, work, psum, whh_t, pre, hbuf, L, n, nh4, KC, rev):
    H = nh4 // 4
    MC = nh4 // 128
    ng = MC // 4
    c_t = work.tile([128, ng, n], F32, tag="lc")
    h_t = work.tile([128, ng, n], BF16, tag="lh")
    nc.vector.memset(c_t[:], 0.0)
    nc.vector.memset(h_t[:], 0.0)
    gsb = work.tile([128, MC, n], F32, tag="lg")
    steps = range(L - 1, -1, -1) if rev else range(L)
    slot = 64 if n <= 64 else 256
    for l in steps:
        ps = psum.tile([128, MC, slot], F32, tag="lps")
        for m in range(MC):
            for k in range(KC):
                nc.tensor.matmul(ps[:, m, :n], whh_t[m * KC + k][:],
                                 h_t[:, k, :],
                                 start=(k == 0), stop=(k == KC - 1))
        for m in range(MC):
            nc.vector.tensor_tensor(out=gsb[:, m, :], in0=ps[:, m, :n],
                                    in1=pre[m][:, l, :], op=OP.add)
        nc.scalar.activation(out=gsb[:, 0:2 * ng, :], in_=gsb[:, 0:2 * ng, :],
                             func=AF.Sigmoid)
        nc.scalar.activation(out=gsb[:, 2 * ng:3 * ng, :],
                             in_=gsb[:, 2 * ng:3 * ng, :], func=AF.Tanh)
        nc.scalar.activation(out=gsb[:, 3 * ng:, :], in_=gsb[:, 3 * ng:, :],
                             func=AF.Sigmoid)
        i_g, f_g = gsb[:, 0:ng, :], gsb[:, ng:2 * ng, :]
        g_g, o_g = gsb[:, 2 * ng:3 * ng, :], gsb[:, 3 * ng:4 * ng, :]
        nc.vector.tensor_tensor(out=c_t[:], in0=f_g, in1=c_t[:], op=OP.mult)
        nc.vector.tensor_tensor(out=i_g, in0=i_g, in1=g_g, op=OP.mult)
        nc.vector.tensor_tensor(out=c_t[:], in0=c_t[:], in1=i_g, op=OP.add)
        tct = work.tile([128, ng, n], F32, tag="ltc")
        nc.scalar.activation(out=tct[:], in_=c_t[:], func=AF.Tanh)
        nc.vector.tensor_tensor(out=h_t[:], in0=o_g, in1=tct[:], op=OP.mult)
        for k in range(KC):
            nc.vector.tensor_copy(out=hbuf[k][:, l, :], in_=h_t[:, k, :])


def build_lstm_launch(which):
    """which: 'intra' or 'inter'. Returns compiled nc."""
    intra = which == "intra"
    ND = 2 if intra else 1
    MC = 4 if intra else 8
    KC = 1 if intra else 2
    NH4 = 512 if intra else 1024
    L = L1 if intra else L2
    NB = TSH if intra else RW2        # lstm batch per core
    NT = L * NB                       # 4250
    ZC = G1 * 128 if intra else RW2 * TP
    G = G1

    nc = new_nc()
    x_pm = nc.dram_tensor("x_pm", [128, G, C], F32, kind="ExternalInput")
    x_u = nc.dram_tensor("x_u", [128, 2, NT], F32, kind="ExternalInput")
    wih = nc.dram_tensor("wih", [64, ND, MC, 4, 128], BF16,
                         kind="ExternalInput")
    whh = nc.dram_tensor("whh", [128, ND, MC * KC, 128], BF16,
                         kind="ExternalInput")
    bih = nc.dram_tensor("bih", [128, ND, MC], F32, kind="ExternalInput")
    ctw = nc.dram_tensor("ctw", [128, ND, 2, KC, 128], BF16,
                         kind="ExternalInput")
    ctb = nc.dram_tensor("ctb", [128, 2], F32, kind="ExternalInput")
    outu = nc.dram_tensor("outu", [128, 2, NT], F32, kind="ExternalOutput")

    ctx = contextlib.ExitStack()
    with tile.TileContext(nc) as tc, ctx:
        const = ctx.enter_context(tc.tile_pool(name="const", bufs=1))
        big = ctx.enter_context(tc.tile_pool(name="big", bufs=1))
        work = ctx.enter_context(tc.tile_pool(name="work", bufs=1))
        psum = ctx.enter_context(tc.tile_pool(name="psum", bufs=2, space="PSUM"))
        psumB = ctx.enter_context(tc.tile_pool(name="psumB", bufs=1,
                                               space="PSUM"))

        eps_t = const.tile([128, 1], F32)
        nc.vector.memset(eps_t[:], EPS)
        ident = const.tile([128, 128], BF16)
        make_identity(nc, ident[:])

        xpm = big.tile([128, G, C], F32, tag="xpm")
        nc.sync.dma_start(out=xpm[:], in_=x_pm[:])
        zpm = ln_posmajor(nc, big, work, xpm, G, C, eps_t)

        # z_cm [64, ZC]; intra: pos=t*68+q ; inter: cols row*1003 + (t+3)
        z_cm = big.tile([C, ZC], BF16, tag="zcm")
        if not intra:
            nc.vector.memset(z_cm[:], 0.0)  # covers causal pad cols
        for g in range(G):
            pt = psum.tile([C, 128], BF16, tag="tps")
            nc.tensor.transpose(pt[:], zpm[:, g, :], ident[:])
            if intra:
                nc.scalar.copy(out=z_cm[:, g * 128:(g + 1) * 128], in_=pt[:])
            else:
                # pos = row*1000 + t -> col row*1003 + t + 3
                p0 = g * 128
                left = min(128, RW2 * T - p0)
                done = 0
                while done < left:
                    pos = p0 + done
                    row, t0 = pos // T, pos % T
                    nn_ = min(left - done, T - t0)
                    nc.scalar.copy(
                        out=z_cm[:, row * TP + 3 + t0:row * TP + 3 + t0 + nn_],
                        in_=pt[:, done:done + nn_])
                    done += nn_

        wih_t = const.tile([64, ND, MC, 4, 128], BF16)
        nc.sync.dma_start(out=wih_t[:], in_=wih[:])
        bih_t = const.tile([128, ND, MC], F32)
        nc.sync.dma_start(out=bih_t[:], in_=bih[:])
        whh_tl = const.tile([128, ND, MC * KC, 128], BF16)
        nc.sync.dma_start(out=whh_tl[:], in_=whh[:])
        ct_tl = const.tile([128, ND, 2, KC, 128], BF16)
        nc.sync.dma_start(out=ct_tl[:], in_=ctw[:])
        ctb_t = const.tile([128, 2], F32)
        nc.sync.dma_start(out=ctb_t[:], in_=ctb[:])
        xu_t = big.tile([128, 2, NT], F32, tag="xut")
        nc.sync.dma_start(out=xu_t[:], in_=x_u[:])

        ysb = big.tile([128, 2, L, NB], F32, tag="xpm")
        hbufs_all = []
        for d in range(ND):
            pre_t = big.tile([128, MC, L, NB], BF16, tag="xpm")
            pre = [pre_t[:, m] for m in range(MC)]
            for m in range(MC):
                for l in range(L):
                    ps = psum.tile([128, 512], F32, tag="ppre")
                    for k in range(4):
                        st = Qp if intra else TP
                        o0 = 4 * l + k
                        rhs = z_cm[:, o0:o0 + st * (NB - 1) + 1:st]
                        nc.tensor.matmul(ps[:, :NB],
                                         wih_t[:, d, m, k, :], rhs,
                                         start=(k == 0), stop=(k == 3))
                    nc.vector.tensor_scalar_add(out=pre[m][:, l, :],
                                                in0=ps[:, :NB],
                                                scalar1=bih_t[:, d, m:m + 1])
            hbuf = [big.tile([128, L, NB], BF16, tag=f"hb{d}_{k}",
                             name=f"hb{d}_{k}") for k in range(KC)]
            lstm(nc, work, psumB,
                 [whh_tl[:, d, i, :] for i in range(MC * KC)],
                 pre, hbuf, L, NB, NH4, KC, rev=(intra and d == 1))
            hbufs_all.append(hbuf)
        for mo in range(2):
            for l in range(L):
                ps2 = psum.tile([128, 512], F32, tag="pct")
                nch = 0
                for d in range(ND):
                    for k in range(KC):
                        nc.tensor.matmul(ps2[:, :NB],
                                         ct_tl[:, d, mo, k, :],
                                         hbufs_all[d][k][:, l, :],
                                         start=(nch == 0),
                                         stop=(nch == ND * KC - 1))
                        nch += 1
                nc.vector.tensor_copy(out=ysb[:, mo, l, :], in_=ps2[:, :NB])
        ou = big.tile([128, 2, NT], F32, tag="ou")
        for mo in range(2):
            nc.vector.scalar_tensor_tensor(
                out=ou[:, mo, :],
                in0=ysb[:, mo].rearrange("p l t -> p (l t)"),
                scalar=ctb_t[:, mo:mo + 1], in1=xu_t[:, mo, :],
                op0=OP.add, op1=OP.add)
        nc.sync.dma_start(out=outu[:], in_=ou[:])
    nc.compile()
    return nc


# ---------------- Launch 3a: QKV conv + PReLU + LN ----------------

def build_l3a():
    nc = new_nc()
    icm = nc.dram_tensor("icm", [64, TSH, Qp], BF16, kind="ExternalInput")
    wall = nc.dram_tensor("wall", [64, 96], BF16, kind="ExternalInput")
    bs = nc.dram_tensor("bs", [96, 4], F32, kind="ExternalInput")
    # bs cols: bias, alpha, cnt_inv, gscale (per row)
    gmat = nc.dram_tensor("gmat", [96, 96], BF16, kind="ExternalInput")
    qkvo = nc.dram_tensor("qkvo", [96, TSH, Qp], BF16, kind="ExternalOutput")
    NTF = TSH * Qp  # 17000
    ctx = contextlib.ExitStack()
    with tile.TileContext(nc) as tc, ctx:
        const = ctx.enter_context(tc.tile_pool(name="const", bufs=1))
        big = ctx.enter_context(tc.tile_pool(name="big", bufs=1))
        work = ctx.enter_context(tc.tile_pool(name="work", bufs=2))
        psum = ctx.enter_context(tc.tile_pool(name="psum", bufs=2, space="PSUM"))
        eps_t = const.tile([96, 1], F32)
        nc.vector.memset(eps_t[:], EPS)
        ict = big.tile([64, NTF], BF16, tag="ict")
        nc.sync.dma_start(out=ict[:], in_=icm.rearrange("c t f -> c (t f)"))
        wt = const.tile([64, 96], BF16)
        nc.sync.dma_start(out=wt[:], in_=wall[:])
        bst = const.tile([96, 4], F32)
        nc.sync.dma_start(out=bst[:], in_=bs[:])
        gm = const.tile([96, 96], BF16)
        nc.sync.dma_start(out=gm[:], in_=gmat[:])

        qr = big.tile([96, NTF], F32, tag="qr")
        for n0 in range(0, NTF, 512):
            nn_ = min(512, NTF - n0)
            ps = psum.tile([96, 512], F32, tag="pc")
            nc.tensor.matmul(ps[:, :nn_], wt[:], ict[:, n0:n0 + nn_],
                             start=True, stop=True)
            nc.vector.tensor_scalar_add(out=qr[:, n0:n0 + nn_],
                                        in0=ps[:, :nn_], scalar1=bst[:, 0:1])
            nc.vector.scalar_tensor_tensor(out=qr[:, n0:n0 + nn_],
                                           in0=qr[:, n0:n0 + nn_],
                                           scalar=bst[:, 1:2],
                                           in1=qr[:, n0:n0 + nn_],
                                           op0=OP.mult, op1=OP.max)
        # stats over (e,f) groups: reduce f, then group-collapse via gmat
        s1 = work.tile([96, TSH], F32, tag="s1")
        nc.vector.tensor_reduce(out=s1[:], in_=qr[:].rearrange(
            "p (t f) -> p t f", f=Qp), axis=AX.X, op=OP.add)
        sq = big.tile([96, NTF], BF16, tag="sq")
        nc.scalar.activation(out=sq[:], in_=qr[:], func=AF.Square)
        s2 = work.tile([96, TSH], F32, tag="s2")
        nc.vector.tensor_reduce(out=s2[:], in_=sq[:].rearrange(
            "p (t f) -> p t f", f=Qp), axis=AX.X, op=OP.add)
        s1b = work.tile([96, TSH], BF16, tag="s1b")
        nc.vector.tensor_copy(out=s1b[:], in_=s1[:])
        s2b = work.tile([96, TSH], BF16, tag="s2b")
        nc.vector.tensor_copy(out=s2b[:], in_=s2[:])
        mu = work.tile([96, TSH], F32, tag="mu")
        ps1 = psum.tile([96, TSH], F32, tag="pg1")
        nc.tensor.matmul(ps1[:], gm[:], s1b[:], start=True, stop=True)
        nc.vector.tensor_scalar_mul(out=mu[:], in0=ps1[:], scalar1=bst[:, 2:3])
        var = work.tile([96, TSH], F32, tag="var")
        ps2g = psum.tile([96, TSH], F32, tag="pg2")
        nc.tensor.matmul(ps2g[:], gm[:], s2b[:], start=True, stop=True)
        nc.vector.tensor_scalar_mul(out=var[:], in0=ps2g[:], scalar1=bst[:, 2:3])
        mu2 = work.tile([96, TSH], F32, tag="mu2")
        nc.vector.tensor_tensor(out=mu2[:], in0=mu[:], in1=mu[:], op=OP.mult)
        nc.vector.tensor_tensor(out=var[:], in0=var[:], in1=mu2[:],
                                op=OP.subtract)
        rs = work.tile([96, TSH], F32, tag="rs")
        nc.scalar.activation(out=rs[:], in_=var[:], func=AF.Sqrt, bias=eps_t[:])
        nc.vector.reciprocal(out=rs[:], in_=rs[:])
        nc.vector.tensor_scalar_mul(out=rs[:], in0=rs[:], scalar1=bst[:, 3:4])
        zh = big.tile([96, TSH, Qp], BF16, tag="zh")
        qr3 = qr[:].rearrange("p (t f) -> p t f", f=Qp)
        nc.vector.tensor_tensor(out=zh[:], in0=qr3, in1=bap(mu, [Qp]),
                                op=OP.subtract)
        nc.vector.tensor_tensor(out=zh[:], in0=zh[:], in1=bap(rs, [Qp]),
                                op=OP.mult)
        nc.vector.memset(zh[:, :, Q:Qp], 0.0)
        nc.sync.dma_start(out=qkvo[:], in_=zh[:])
    nc.compile()
    return nc


# ---------------- Launch 3b: attention per (h,b) ----------------

def build_l3b():
    nc = new_nc()
    # feature-major Q,K (host-transposed): [384, 1000] rows = 272QT+pad, etc
    qT = nc.dram_tensor("qT", [128, 3, T], BF16, kind="ExternalInput")
    kT = nc.dram_tensor("kT", [128, 3, T], BF16, kind="ExternalInput")
    vm = nc.dram_tensor("vm", [128, 8, Dv * Qp], BF16,
                        kind="ExternalInput")
    msk = nc.dram_tensor("msk", [128, 128], F32, kind="ExternalInput")
    avo = nc.dram_tensor("avo", [128, 8, Dv * Qp], BF16,
                         kind="ExternalOutput")
    DFv = Dv * Qp
    ctx = contextlib.ExitStack()
    with tile.TileContext(nc) as tc, ctx:
        const = ctx.enter_context(tc.tile_pool(name="const", bufs=1))
        big = ctx.enter_context(tc.tile_pool(name="big", bufs=1))
        work = ctx.enter_context(tc.tile_pool(name="work", bufs=3))
        psum = ctx.enter_context(tc.tile_pool(name="psum", bufs=2, space="PSUM"))
        psumB = ctx.enter_context(tc.tile_pool(name="psumB", bufs=1,
                                               space="PSUM"))
        ident = const.tile([128, 128], F32)
        make_identity(nc, ident[:])
        qt_t = big.tile([128, 3, T], BF16, tag="qt")
        nc.sync.dma_start(out=qt_t[:], in_=qT[:])
        kt_t = big.tile([128, 3, T], BF16, tag="kt")
        nc.sync.dma_start(out=kt_t[:], in_=kT[:])
        vm_t = big.tile([128, 8, DFv], BF16, tag="vm")
        nc.sync.dma_start(out=vm_t[:], in_=vm[:])
        msk_t = const.tile([128, 128], F32)
        nc.sync.dma_start(out=msk_t[:], in_=msk[:])

        for tcn in range(8):
            ns = min((tcn + 1) * 128, T)
            tch = min(128, T - tcn * 128)
            sc = big.tile([128, 1024], F32, tag="sc")
            for s0 in range(0, ns, 512):
                nn_ = min(512, ns - s0)
                ps = psum.tile([128, 512], F32, tag="psc")
                for kc in range(3):
                    nc.tensor.matmul(
                        ps[:tch, :nn_],
                        qt_t[:, kc, tcn * 128:tcn * 128 + tch],
                        kt_t[:, kc, s0:s0 + nn_],
                        start=(kc == 0), stop=(kc == 2))
                nc.vector.tensor_copy(out=sc[:tch, s0:s0 + nn_],
                                      in_=ps[:tch, :nn_])
            dw = ns - tcn * 128
            nc.vector.tensor_tensor(out=sc[:tch, tcn * 128:ns],
                                    in0=sc[:tch, tcn * 128:ns],
                                    in1=msk_t[:tch, :dw], op=OP.add)
            mx = work.tile([128, 1], F32, tag="mx")
            nc.vector.tensor_reduce(out=mx[:tch], in_=sc[:tch, :ns], axis=AX.X,
                                    op=OP.max)
            nc.vector.tensor_scalar_mul(out=mx[:tch], in0=mx[:tch],
                                        scalar1=-1.0)
            sme = work.tile([128, 1], F32, tag="sme")
            nc.scalar.activation(out=sc[:tch, :ns], in_=sc[:tch, :ns],
                                 func=AF.Exp, bias=mx[:tch],
                                 accum_out=sme[:tch])
            nc.vector.reciprocal(out=sme[:tch], in_=sme[:tch])
            av = psumB.tile([128, 3, 512], F32, tag="pav")
            for sb in range(tcn + 1):
                scb = min(128, ns - sb * 128)
                pT = psum.tile([128, 128], F32, tag="ptr")
                nc.tensor.transpose(pT[:scb, :tch],
                                    sc[:tch, sb * 128:sb * 128 + scb],
                                    ident[:tch, :tch])
                aT = work.tile([128, 128], BF16, tag="aT")
                nc.scalar.copy(out=aT[:scb, :tch], in_=pT[:scb, :tch])
                for n3 in range(3):
                    nn_ = min(512, DFv - n3 * 512)
                    nc.tensor.matmul(av[:tch, n3, :nn_], aT[:scb, :tch],
                                     vm_t[:scb, sb, n3 * 512:n3 * 512 + nn_],
                                     start=(sb == 0), stop=(sb == tcn))
            avs = big.tile([128, DFv], BF16, tag="avs")
            av2 = bass.AP(tensor=av.tensor, offset=av.offset,
                          ap=[av.ap[0], [1, DFv]])
            nc.vector.tensor_scalar_mul(out=avs[:tch], in0=av2[:tch],
                                        scalar1=sme[:tch])
            nc.sync.dma_start(out=avo[:, tcn, :], in_=avs[:])
    nc.compile()
    return nc


# ---------------- Launch 3c: proj + out-LN + residual ----------------

def build_l3c():
    nc = new_nc()
    avf = nc.dram_tensor("avf", [64, TSH, Qp], BF16, kind="ExternalInput")
    icm = nc.dram_tensor("icm", [64, TSH, Qp], F32, kind="ExternalInput")
    pw = nc.dram_tensor("pw", [64, 64], BF16, kind="ExternalInput")
    pb = nc.dram_tensor("pb", [64, 3], F32, kind="ExternalInput")
    # pb cols: bias, gamma0*? , ... col0 bias, col1 alpha-scalar bcast
    outo = nc.dram_tensor("outo", [64, TSH, Q], F32, kind="ExternalOutput")
    NTF = TSH * Qp
    ctx = contextlib.ExitStack()
    with tile.TileContext(nc) as tc, ctx:
        const = ctx.enter_context(tc.tile_pool(name="const", bufs=1))
        big = ctx.enter_context(tc.tile_pool(name="big", bufs=1))
        work = ctx.enter_context(tc.tile_pool(name="work", bufs=1))
        psum = ctx.enter_context(tc.tile_pool(name="psum", bufs=2, space="PSUM"))
        eps_t = const.tile([128, 1], F32)
        nc.vector.memset(eps_t[:], EPS)
        ones_t = const.tile([64, 128], BF16)
        nc.vector.memset(ones_t[:], 1.0)
        avt = big.tile([64, NTF], BF16, tag="avt")
        nc.sync.dma_start(out=avt[:], in_=avf.rearrange("c t f -> c (t f)"))
        pwt = const.tile([64, 64], BF16)
        nc.sync.dma_start(out=pwt[:], in_=pw[:])
        pbt = const.tile([64, 3], F32)
        nc.sync.dma_start(out=pbt[:], in_=pb[:])

        P = big.tile([64, NTF], F32, tag="P")
        for n0 in range(0, NTF, 512):
            nn_ = min(512, NTF - n0)
            ps = psum.tile([64, 512], F32, tag="pp")
            nc.tensor.matmul(ps[:, :nn_], pwt[:], avt[:, n0:n0 + nn_],
                             start=True, stop=True)
            nc.vector.tensor_scalar_add(out=P[:, n0:n0 + nn_],
                                        in0=ps[:, :nn_], scalar1=pbt[:, 0:1])
            nc.vector.scalar_tensor_tensor(out=P[:, n0:n0 + nn_],
                                           in0=P[:, n0:n0 + nn_],
                                           scalar=pbt[:, 1:2],
                                           in1=P[:, n0:n0 + nn_], op0=OP.mult,
                                           op1=OP.max)
        P3 = P[:].rearrange("p (t f) -> p t f", f=Qp)
        nc.vector.memset(P3[:, :, Q:Qp], 0.0)
        s1 = work.tile([64, TSH], F32, tag="s1")
        nc.vector.tensor_reduce(out=s1[:], in_=P3, axis=AX.X, op=OP.add)
        sq = big.tile([64, NTF], BF16, tag="avt")
        nc.scalar.activation(out=sq[:], in_=P[:], func=AF.Square)
        s2 = work.tile([64, TSH], F32, tag="s2")
        nc.vector.tensor_reduce(out=s2[:], in_=sq[:].rearrange(
            "p (t f) -> p t f", f=Qp), axis=AX.X, op=OP.add)
        s1b = work.tile([64, TSH], BF16, tag="s1b")
        nc.vector.tensor_copy(out=s1b[:], in_=s1[:])
        s2b = work.tile([64, TSH], BF16, tag="s2b")
        nc.vector.tensor_copy(out=s2b[:], in_=s2[:])
        NCF = 64 * Q  # 4160
        mu = work.tile([128, TSH], F32, tag="mu")
        psg = psum.tile([128, TSH], F32, tag="pg")
        nc.tensor.matmul(psg[:], ones_t[:], s1b[:], start=True, stop=True)
        nc.vector.tensor_scalar_mul(out=mu[:], in0=psg[:], scalar1=1.0 / NCF)
        var = work.tile([128, TSH], F32, tag="var")
        psg2 = psum.tile([128, TSH], F32, tag="pg2")
        nc.tensor.matmul(psg2[:], ones_t[:], s2b[:], start=True, stop=True)
        nc.vector.tensor_scalar_mul(out=var[:], in0=psg2[:], scalar1=1.0 / NCF)
        mu2 = work.tile([128, TSH], F32, tag="mu2")
        nc.vector.tensor_tensor(out=mu2[:], in0=mu[:], in1=mu[:], op=OP.mult)
        nc.vector.tensor_tensor(out=var[:], in0=var[:], in1=mu2[:],
                                op=OP.subtract)
        rs = work.tile([128, TSH], F32, tag="rs")
        nc.scalar.activation(out=rs[:], in_=var[:], func=AF.Sqrt, bias=eps_t[:])
        nc.vector.reciprocal(out=rs[:], in_=rs[:])
        # out = (P - mu)*rs + inter
        o1 = big.tile([64, TSH, Qp], F32, tag="o1")
        nc.vector.tensor_tensor(out=o1[:], in0=P3, in1=bap(mu[0:64, :], [Qp]),
                                op=OP.subtract)
        nc.vector.tensor_tensor(out=o1[:], in0=o1[:], in1=bap(rs[0:64, :], [Qp]),
                                op=OP.mult)
        ict = big.tile([64, NTF], F32, tag="P")
        nc.sync.dma_start(out=ict[:], in_=icm.rearrange("c t f -> c (t f)"))
        nc.vector.tensor_tensor(out=o1[:], in0=o1[:],
                                in1=ict[:].rearrange("p (t f) -> p t f", f=Qp),
                                op=OP.add)
        nc.sync.dma_start(out=outo[:], in_=o1[:, :, :Q])
    nc.compile()
    return nc


# ======================= host side =======================

_CACHE = {}


def _posmajor(arr_pos_c, G):
    """[NPOS, nred] -> [128, G, nred] tiles, pos = g*128+p."""
    npos, nred = arr_pos_c.shape
    pad = np.zeros((G * 128, nred), arr_pos_c.dtype)
    pad[:npos] = arr_pos_c
    return np.ascontiguousarray(pad.reshape(G, 128, nred).transpose(1, 0, 2))


def _lstm_weight_prep(wih, whh, bih, bhh, ctw, ctb, gamma, beta, MC, KC):
    """Fold LN gamma/beta into wih/bias; build device layouts."""
    g = gamma.reshape(-1).astype(np.float64)   # [C]
    b = beta.reshape(-1).astype(np.float64)
    wih = np.asarray(wih, np.float64)          # [4H, C*KS]
    NH4 = wih.shape[0]
    w4 = wih.reshape(NH4, C, KS)
    wih_eff = w4 * g[None, :, None]
    bih_eff = (np.asarray(bih, np.float64) + np.asarray(bhh, np.float64)
               + (w4 * b[None, :, None]).sum((1, 2)))
    # device wih tile [MC, 4, 64, 128]: [m, k, c, gate-in-chunk]
    wt = np.zeros((MC, 4, 64, 128), np.float32)
    for m in range(MC):
        for k in range(4):
            wt[m, k] = wih_eff[m * 128:(m + 1) * 128, :, k].T
    # whh lhsT [MC*KC, 128, 128]: chunk (m,kc): whh[m*128:.., kc*128:..].T
    whh = np.asarray(whh, np.float64)
    wh = np.zeros((MC * KC, 128, 128), np.float32)
    for m in range(MC):
        for kc in range(KC):
            wh[m * KC + kc] = whh[m * 128:(m + 1) * 128,
                                  kc * 128:(kc + 1) * 128].T
    bih_t = np.zeros((128, MC), np.float32)
    for m in range(MC):
        bih_t[:, m] = bih_eff[m * 128:(m + 1) * 128]
    # convT: ctw [HIDd, 64, 4] -> [2, KC*128, 128] ; out rows (k',c) k'*64+c
    ctw = np.asarray(ctw, np.float64)
    KCc = ctw.shape[0] // 128
    ct = np.zeros((2, KCc * 128, 128), np.float32)
    for mo in range(2):
        for kp in range(2):
            for cc in range(64):
                j = kp * 64 + cc
                ct[mo, :, j] = ctw[:, cc, mo * 2 + kp]
    ctb_t = np.zeros((128, 2), np.float32)
    for mo in range(2):
        for kp in range(2):
            ctb_t[kp * 64:(kp + 1) * 64, mo] = np.asarray(ctb)
    return wt, wh, bih_t, ct, ctb_t


def _unf_rows(arr_c_t, L, off=0):
    """arr [64, NTIME] -> x_u [2, 128, L, NB] rows (k,c) k*64+c, cols (l, nb).
    value = arr[c, nb, 4l+k+off] where arr is [64, NB, NTIME-per-row]."""
    C_, NB, NT_ = arr_c_t.shape
    out = np.zeros((2, 128, L, NB), np.float32)
    for mo in range(2):
        for kp in range(2):
            k = mo * 2 + kp
            idx = 4 * np.arange(L) + k + off
            v = arr_c_t[:, :, :][:, :, idx]          # [64, NB, L]
            out[mo, kp * 64:(kp + 1) * 64] = v.transpose(0, 2, 1)
    return out


def _uniform(a):
    a = np.asarray(a)
    assert np.all(a == a.flat[0]), "nonuniform LN affine not supported"
    return float(a.flat[0])


def kernel(**inputs):
    ii = {k: np.asarray(v) for k, v in inputs.items()}
    x = ii["x"].astype(np.float32)
    xp = np.zeros((B, C, T, Qp), np.float32)
    xp[:, :, :, :Q] = x

    if "l1" not in _CACHE:
        _CACHE["l1"] = build_lstm_launch("intra")
        _CACHE["l2"] = build_lstm_launch("inter")
        _CACHE["l3a"] = build_l3a()
        _CACHE["l3b"] = build_l3b()
        _CACHE["l3c"] = build_l3c()

    bf = lambda a: np.ascontiguousarray(a, dtype=np.float32).astype(
        mybir.dt.np(BF16))
    f32c = lambda a: np.ascontiguousarray(a, dtype=np.float32)

    # ---------- L1 ----------
    wt, wh, bih_t, ct, ctb_t = _lstm_weight_prep(
        ii["intra_wih"][0], ii["intra_whh"][0], ii["intra_bih"][0],
        ii["intra_bhh"][0], None, None, None, None, 4, 1) if False else (None,) * 5
    # fw and bw separately (dirs stacked)
    wts, whs, bihs = [], [], []
    for d in range(2):
        a, b_, c_, _, _ = _lstm_weight_prep(
            ii["intra_wih"][d], ii["intra_whh"][d], ii["intra_bih"][d],
            ii["intra_bhh"][d], ii["intra_ct_w"], ii["intra_ct_b"],
            ii["intra_gamma"], ii["intra_beta"], 4, 1)
        wts.append(a); whs.append(b_); bihs.append(c_)
    _, _, _, ct1, ctb1 = _lstm_weight_prep(
        ii["intra_wih"][0], ii["intra_whh"][0], ii["intra_bih"][0],
        ii["intra_bhh"][0], ii["intra_ct_w"], ii["intra_ct_b"],
        ii["intra_gamma"], ii["intra_beta"], 4, 1)
    # intra ctw [256,64,4]: split fw rows 0:128, bw 128:256 across d
    ctw_i = np.asarray(ii["intra_ct_w"], np.float64)
    ct_d = np.zeros((2, 2, 128, 128), np.float32)
    for d in range(2):
        sub = ctw_i[d * 128:(d + 1) * 128]
        for mo in range(2):
            for kp in range(2):
                for cc in range(64):
                    ct_d[d, mo, :, kp * 64 + cc] = sub[:, cc, mo * 2 + kp]
    l1_w = {
        "wih": bf(np.stack(wts).transpose(3, 0, 1, 2, 4)),
        "whh": bf(np.stack(whs).transpose(2, 0, 1, 3)),
        "bih": f32c(np.stack(bihs, axis=1)),
        "ctw": bf(ct_d.reshape(2, 2, 1, 128, 128).transpose(3, 0, 1, 2, 4)),
        "ctb": f32c(ctb1),
    }
    l1_maps = []
    for core in range(NCORES):
        b = core // 4
        t0 = (core % 4) * TSH
        xs = xp[b, :, t0:t0 + TSH, :]                    # [C, TSH, Qp]
        x_pm = _posmajor(np.ascontiguousarray(
            xs.transpose(1, 2, 0)).reshape(NP1, C), G1)
        xu = _unf_rows(xs.transpose(0, 1, 2).reshape(C, TSH, Qp)
                       .transpose(0, 1, 2), L1)          # wait: per row=t
        # arr [64, NB=TSH, Qp]
        xu = _unf_rows(np.ascontiguousarray(xs.transpose(0, 1, 2)), L1)
        l1_maps.append({**l1_w, "x_pm": x_pm,
                        "x_u": f32c(xu.reshape(2, 128, L1 * TSH)
                                    .transpose(1, 0, 2))})
    r1 = run_bass_kernel_spmd(_CACHE["l1"], l1_maps,
                              core_ids=list(range(NCORES))).results
    # reassemble intra [B, C, T, Qp]
    intra = np.zeros((B, C, T, Qp), np.float32)
    for core in range(NCORES):
        b = core // 4
        t0 = (core % 4) * TSH
        ou = r1[core]["outu"].transpose(1, 0, 2).reshape(2, 128, L1, TSH)
        for mo in range(2):
            for kp in range(2):
                k = mo * 2 + kp
                q_idx = 4 * np.arange(L1) + k
                intra[b, :, t0:t0 + TSH, q_idx] = \
                    ou[mo, kp * 64:(kp + 1) * 64].transpose(1, 0, 2)
    # ---------- L2 ----------
    wts2, whs2, bihs2 = [], [], []
    a, b_, c_, ct2, ctb2 = _lstm_weight_prep(
        ii["inter_wih"], ii["inter_whh"], ii["inter_bih"], ii["inter_bhh"],
        ii["inter_ct_w"], ii["inter_ct_b"], ii["inter_gamma"],
        ii["inter_beta"], 8, 2)
    assert _uniform(ii["inter_beta"]) == 0.0
    ct2_d = ct2.reshape(1, 2, 256, 128)
    l2_w = {"wih": bf(a.transpose(2, 0, 1, 3).reshape(64, 1, 8, 4, 128)),
            "whh": bf(b_.transpose(1, 0, 2).reshape(128, 1, 16, 128)),
            "bih": f32c(c_.reshape(128, 1, 8)),
            "ctw": bf(ct2.reshape(2, 2, 128, 128).transpose(2, 0, 1, 3)
                      .reshape(128, 1, 2, 2, 128)),
            "ctb": f32c(ctb2)}
    l2_maps = []
    for core in range(NCORES):
        b = core // 4
        q0 = (core % 4) * RW2
        isl = intra[b, :, :, q0:q0 + RW2]                # [C, T, RW2]
        rows_ct = np.ascontiguousarray(isl.transpose(0, 2, 1))  # [C,RW2,T]
        x_pm = _posmajor(np.ascontiguousarray(
            rows_ct.transpose(1, 2, 0)).reshape(RW2 * T, C), G1)
        # x_u resid: value = intra[c, row, t=4l+k]
        xu = _unf_rows(rows_ct, L2, off=0)
        l2_maps.append({**l2_w, "x_pm": x_pm,
                        "x_u": f32c(xu.reshape(2, 128, L2 * RW2)
                                    .transpose(1, 0, 2))})
    r2 = run_bass_kernel_spmd(_CACHE["l2"], l2_maps,
                              core_ids=list(range(NCORES))).results
    inter = np.zeros((B, C, T, Qp), np.float32)
    for core in range(NCORES):
        b = core // 4
        q0 = (core % 4) * RW2
        ou = r2[core]["outu"].transpose(1, 0, 2).reshape(2, 128, L2, RW2)
        for mo in range(2):
            for kp in range(2):
                k = mo * 2 + kp
                t_idx = 4 * np.arange(L2) + k
                inter[b, :, t_idx, q0:q0 + RW2] = \
                    ou[mo, kp * 64:(kp + 1) * 64].transpose(1, 0, 2)
    inter_r = np.zeros((B, C, T, Qp), np.float32)
    inter_r[:, :, :, :Q] = inter[:, :, :, :Q]            # real freqs only
    # ---------- L3a ----------
    qg = _uniform(ii["q_g"]); kg = _uniform(ii["k_g"]); vg = _uniform(ii["v_g"])
    assert _uniform(ii["q_bt"]) == 0 and _uniform(ii["k_bt"]) == 0
    assert _uniform(ii["v_bt"]) == 0
    wall = np.zeros((64, 96), np.float32)
    bias96 = np.zeros((96,), np.float32)
    alpha96 = np.zeros((96,), np.float32)
    cnt96 = np.zeros((96,), np.float32)
    gs96 = np.zeros((96,), np.float32)
    grp = np.zeros((96,), np.int32)
    for h in range(NH):
        wall[:, h * 4:h * 4 + 4] = np.asarray(ii["q_w"][h]).T
        wall[:, 16 + h * 4:16 + h * 4 + 4] = np.asarray(ii["k_w"][h]).T
        wall[:, 32 + h * 16:32 + h * 16 + 16] = np.asarray(ii["v_w"][h]).T
        bias96[h * 4:h * 4 + 4] = np.asarray(ii["q_b"][h])
        bias96[16 + h * 4:16 + h * 4 + 4] = np.asarray(ii["k_b"][h])
        alpha96[h * 4:h * 4 + 4] = float(ii["q_p"][h])
        alpha96[16 + h * 4:16 + h * 4 + 4] = float(ii["k_p"][h])
        alpha96[32 + h * 16:32 + h * 16 + 16] = float(ii["v_p"][h])
        cnt96[h * 4:h * 4 + 4] = 1.0 / (E * Q)
        cnt96[16 + h * 4:16 + h * 4 + 4] = 1.0 / (E * Q)
        cnt96[32 + h * 16:32 + h * 16 + 16] = 1.0 / (Dv * Q)
        gs96[h * 4:h * 4 + 4] = qg / np.sqrt(E * Q)
        gs96[16 + h * 4:16 + h * 4 + 4] = kg
        gs96[32 + h * 16:32 + h * 16 + 16] = vg
        grp[h * 4:h * 4 + 4] = h
        grp[16 + h * 4:16 + h * 4 + 4] = 4 + h
        grp[32 + h * 16:32 + h * 16 + 16] = 8 + h
    gmat = (grp[:, None] == grp[None, :]).astype(np.float32)
    bs96 = np.stack([bias96, alpha96, cnt96, gs96], axis=1)
    l3a_w = {"wall": bf(wall), "bs": f32c(bs96), "gmat": bf(gmat)}
    l3a_maps = []
    for core in range(NCORES):
        b = core // 4
        t0 = (core % 4) * TSH
        l3a_maps.append({**l3a_w,
                         "icm": bf(inter_r[b, :, t0:t0 + TSH, :])})
    r3a = run_bass_kernel_spmd(_CACHE["l3a"], l3a_maps,
                               core_ids=list(range(NCORES))).results
    qkv = np.zeros((B, 96, T, Qp), np.float32)
    for core in range(NCORES):
        b = core // 4
        t0 = (core % 4) * TSH
        qkv[b, :, t0:t0 + TSH, :] = r3a[core]["qkvo"].astype(
            np.float32).transpose(0, 1, 2)
    # ---------- L3b ----------
    mask = np.triu(np.full((128, 128), -1e9, np.float32), 1)
    l3b_maps = []
    for core in range(NCORES):
        h, b = core % 4, core // 4
        qh = qkv[b, h * 4:h * 4 + 4]                     # [4, T, Qp]
        kh = qkv[b, 16 + h * 4:16 + h * 4 + 4]
        vh = qkv[b, 32 + h * 16:32 + h * 16 + 16]        # [16, T, Qp]
        qT = np.zeros((384, T), np.float32)
        kT = np.zeros((384, T), np.float32)
        qT[:4 * Qp] = qh.transpose(0, 2, 1).reshape(4 * Qp, T)
        kT[:4 * Qp] = kh.transpose(0, 2, 1).reshape(4 * Qp, T)
        vm = np.zeros((8, 128, Dv * Qp), np.float32)
        vflat = vh.transpose(1, 0, 2).reshape(T, Dv * Qp)  # [s, (d,f)]
        vm.reshape(1024, Dv * Qp)[:T] = vflat
        l3b_maps.append({"qT": bf(qT.reshape(3, 128, T).transpose(1, 0, 2)),
                         "kT": bf(kT.reshape(3, 128, T).transpose(1, 0, 2)),
                         "vm": bf(vm.transpose(1, 0, 2)), "msk": f32c(mask)})
    r3b = run_bass_kernel_spmd(_CACHE["l3b"], l3b_maps,
                               core_ids=list(range(NCORES))).results
    # av: [B, (h,d) 64, T, Qp]
    av = np.zeros((B, 64, T, Qp), np.float32)
    for core in range(NCORES):
        h, b = core % 4, core // 4
        a_ = r3b[core]["avo"].astype(np.float32).transpose(1, 0, 2)\
            .reshape(1024, Dv, Qp)[:T]
        av[b, h * 16:(h + 1) * 16] = a_.transpose(1, 0, 2)
    # ---------- L3c ----------
    assert _uniform(ii["proj_g"]) == 1.0 and _uniform(ii["proj_bt"]) == 0.0
    pw = np.asarray(ii["proj_w"], np.float32).T          # lhsT [hd, c]
    pb3 = np.zeros((64, 3), np.float32)
    pb3[:, 0] = np.asarray(ii["proj_b"])
    pb3[:, 1] = float(ii["proj_p"])
    l3c_w = {"pw": bf(pw), "pb": f32c(pb3)}
    l3c_maps = []
    for core in range(NCORES):
        b = core // 4
        t0 = (core % 4) * TSH
        l3c_maps.append({**l3c_w, "avf": bf(av[b, :, t0:t0 + TSH, :]),
                         "icm": f32c(inter_r[b, :, t0:t0 + TSH, :])})
    r3c = run_bass_kernel_spmd(_CACHE["l3c"], l3c_maps,
                               core_ids=list(range(NCORES))).results
    out = np.zeros((B, C, T, Q), np.float32)
    for core in range(NCORES):
        b = core // 4
        t0 = (core % 4) * TSH
        out[b, :, t0:t0 + TSH, :] = r3c[core]["outo"]
    kernel.dbg = {"intra": intra, "inter": inter, "qkv": qkv, "av": av}
    return out



# revision 16
# speedup vs baseline: 1.0799x; 1.0799x over previous
"""GridNetBlock (TF-GridNet) Trainium2 kernel: 8-core SPMD, 5 launches."""
import sys, os, contextlib
for _p in ("/opt/trn_rl_repo", "/root/.axon_site/_ro/trn_rl_repo"):
    if os.path.isdir(_p) and _p not in sys.path:
        sys.path.insert(0, _p)
import numpy as np
import concourse.bass as bass
import concourse.bacc as bacc
import concourse.tile as tile
from concourse import mybir
from concourse.masks import make_identity
from concourse.bass_utils import run_bass_kernel_spmd

F32 = mybir.dt.float32
BF16 = mybir.dt.bfloat16
AF = mybir.ActivationFunctionType
OP = mybir.AluOpType
AX = mybir.AxisListType

B, C, T, Q = 2, 64, 1000, 65
KS = 4
Qp, L1, Hh, HID, L2 = 68, 17, 128, 256, 250
NH, E, Dv = 4, 4, 16
EPS = 1e-5
NCORES = 8
TSH = T // 4
NP1 = TSH * Qp
G1 = (NP1 + 127) // 128   # 133
RW2 = (B * Qp) // NCORES  # 17
TP = 1003                 # causal padded time
NT1 = L1 * TSH            # 4250
NT2 = L2 * RW2            # 4250


def bap(t, tail):
    ap = list(t.ap)
    for n in tail:
        ap.append([0, n])
    return bass.AP(tensor=t.tensor, offset=t.offset, ap=ap)


def shift_ap(t, off, dims):
    return bass.AP(tensor=t.tensor, offset=t.offset + off, ap=[t.ap[0]] + dims)


def new_nc():
    return bacc.Bacc("TRN2", target_bir_lowering=False, debug=False,
                     enable_asserts=True, num_devices=NCORES)


def ln_posmajor(nc, pool, work, xpm, G, nred, eps_t):
    s1 = work.tile([128, G], F32, tag="lns1")
    nc.vector.tensor_reduce(out=s1[:], in_=xpm[:], axis=AX.X, op=OP.add)
    xsq = pool.tile([128, G, nred], BF16, tag="xut")
    nc.scalar.activation(out=xsq[:], in_=xpm[:], func=AF.Square)
    s2 = work.tile([128, G], F32, tag="lns2")
    nc.vector.tensor_reduce(out=s2[:], in_=xsq[:], axis=AX.X, op=OP.add)
    mu = work.tile([128, G], F32, tag="lnmu")
    nc.vector.tensor_scalar_mul(out=mu[:], in0=s1[:], scalar1=1.0 / nred)
    var = work.tile([128, G], F32, tag="lnvar")
    nc.vector.tensor_tensor(out=var[:], in0=mu[:], in1=mu[:], op=OP.mult)
    nc.vector.scalar_tensor_tensor(out=var[:], in0=s2[:], scalar=1.0 / nred,
                                   in1=var[:], op0=OP.mult, op1=OP.subtract)
    rs = work.tile([128, G], F32, tag="lnrs")
    nc.scalar.activation(out=rs[:], in_=var[:], func=AF.Sqrt, bias=eps_t[:])
    nc.vector.reciprocal(out=rs[:], in_=rs[:])
    zpm = pool.tile([128, G, nred], BF16, tag="xut")
    nc.vector.tensor_tensor(out=zpm[:], in0=xpm[:], in1=bap(mu, [nred]),
                            op=OP.subtract)
    nc.vector.tensor_tensor(out=zpm[:], in0=zpm[:], in1=bap(rs, [nred]),
                            op=OP.mult)
    return zpm


def ap3(t, off, d1, n1, d2, n2):
    """Strided 2-free-dim AP view of tile t at element offset off."""
    return bass.AP(tensor=t.tensor, offset=t.offset + off,
                   ap=[t.ap[0], [d1, n1], [d2, n2]])


def build_lstm_launch(which):
    """which: 'intra' or 'inter'. Returns compiled nc."""
    intra = which == "intra"
    ND = 2 if intra else 1
    MC = 4 if intra else 8
    KC = 1 if intra else 2
    L = L1 if intra else L2
    NB = TSH if intra else RW2        # lstm batch per core
    NT = L * NB                       # 4250
    G = G1
    ZC = G * 128

    nc = new_nc()
    x_pm = nc.dram_tensor("x_pm", [128, G, C], F32, kind="ExternalInput")
    x_u = nc.dram_tensor("x_u", [128, 2, NT], F32, kind="ExternalInput")
    if intra:
        wih = nc.dram_tensor("wih", [65, 2, 4, 4, 128], BF16,
                             kind="ExternalInput")
        whh = nc.dram_tensor("whh", [128, 2, 4, 128], BF16,
                             kind="ExternalInput")
    else:
        wih = nc.dram_tensor("wih", [128, 8, 2, 128], BF16,
                             kind="ExternalInput")
        whh = nc.dram_tensor("whh", [128, 16, 128], BF16,
                             kind="ExternalInput")
        bih = nc.dram_tensor("bih", [128, 8], F32, kind="ExternalInput")
    ctw = nc.dram_tensor("ctw", [128, ND, 2, KC, 128], BF16,
                         kind="ExternalInput")
    ctb = nc.dram_tensor("ctb", [128, 2], F32, kind="ExternalInput")
    outu = nc.dram_tensor("outu", [128, 2, NT], F32, kind="ExternalOutput")

    ctx = contextlib.ExitStack()
    with tile.TileContext(nc) as tc, ctx:
        const = ctx.enter_context(tc.tile_pool(name="const", bufs=1))
        persist = ctx.enter_context(tc.tile_pool(name="persist", bufs=1))
        psum = ctx.enter_context(tc.tile_pool(name="psum", bufs=2,
                                              space="PSUM"))
        psumB = ctx.enter_context(tc.tile_pool(name="psumB", bufs=2,
                                               space="PSUM"))
        psumS = ctx.enter_context(tc.tile_pool(
            name="psumS", bufs=1 if intra else 2, space="PSUM"))

        eps_t = const.tile([128, 1], F32)
        nc.vector.memset(eps_t[:], EPS)
        ident = const.tile([128, 128], BF16)
        make_identity(nc, ident[:])

        if intra:
            wih_t = const.tile([65, 2, 4, 4, 128], BF16)
            whh_t = const.tile([128, 2, 4, 128], BF16)
        else:
            wih_t = const.tile([128, 8, 2, 128], BF16)
            whh_t = const.tile([128, 16, 128], BF16)
            bih_t = const.tile([128, 8], F32)
            nc.sync.dma_start(out=bih_t[:], in_=bih[:])
        nc.sync.dma_start(out=wih_t[:], in_=wih[:])
        nc.sync.dma_start(out=whh_t[:], in_=whh[:])
        ct_tl = const.tile([128, ND, 2, KC, 128], BF16)
        nc.sync.dma_start(out=ct_tl[:], in_=ctw[:])
        ctb_t = const.tile([128, 2], F32)
        nc.sync.dma_start(out=ctb_t[:], in_=ctb[:])

        # --- persistent tiles ---
        if intra:
            # z channel-major [65, ZC]; row 64 = ones (bias row)
            z_cm = persist.tile([65, ZC], BF16)
            hbufs = [persist.tile([128, L, NB], BF16, name=f"hb{d}")
                     for d in range(ND * KC)]
        else:
            # z doubled rows: p<64: z[c, t'-3]; p>=64: z[c, t'-2]
            z2d = persist.tile([128, RW2, 1000], BF16)
            hb2 = persist.tile([128, 2, L, NB], BF16)
        ou = persist.tile([128, 2, L, NB], F32)

        # --- LN over C (pos-major) + transpose to channel-major ---
        with tc.tile_pool(name="tmpA", bufs=1) as tmpA:
            xpm = tmpA.tile([128, G, C], F32)
            nc.sync.dma_start(out=xpm[:], in_=x_pm[:])
            work = tmpA
            zpm = ln_posmajor(nc, tmpA, work, xpm, G, C, eps_t)
            if intra:
                z_dst = z_cm
                nc.vector.memset(z_cm[64:65, :], 1.0)
            else:
                z_tmp = tmpA.tile([64, ZC], BF16)
                z_dst = z_tmp
            for g0 in range(0, G, 4):
                gn = min(4, G - g0)
                pt = psum.tile([64, 4, 128], BF16, tag="tps")
                for gg in range(gn):
                    nc.tensor.transpose(pt[:, gg, :], zpm[:, g0 + gg, :],
                                        ident[:])
                nc.scalar.copy(
                    out=z_dst[0:64, g0 * 128:(g0 + gn) * 128],
                    in_=pt[:, 0:gn, :].rearrange("p a b -> p (a b)"))
            if not intra:
                # z_tmp [64, row*1000+t] -> z2d two shifted copies
                nc.vector.memset(z2d[:], 0.0)
                nc.sync.dma_start(
                    out=z2d[0:64, :, 3:1000],
                    in_=ap3(z_tmp, 0, 1000, RW2, 1, 997))
                nc.sync.dma_start(
                    out=z2d[64:128, :, 2:1000],
                    in_=ap3(z_tmp, 0, 1000, RW2, 1, 998))

        # --- gate precompute (inter only): pre2 [128, 8, RW2, L] bf16 ---
        rec = ctx.enter_context(tc.tile_pool(name="rec", bufs=1))
        if not intra:
            pre2 = rec.tile([128, 8, RW2, L], BF16)
            LSUB = 30
            for m in range(8):
                for l0 in range(0, L, LSUB):
                    ln_ = min(LSUB, L - l0)
                    pp = psumB.tile([128, 512], F32, tag="ppre")
                    for kp in range(2):
                        rhs = ap3(z2d, 4 * l0 + 2 * kp, 1000, RW2, 4, ln_)
                        nc.tensor.matmul(pp[:, :RW2 * ln_],
                                         wih_t[:, m, kp, :], rhs,
                                         start=(kp == 0), stop=(kp == 1))
                    nc.vector.tensor_scalar_add(
                        out=pre2[:, m, :, l0:l0 + ln_],
                        in0=pp[:, :RW2 * ln_].rearrange(
                            "p (r l) -> p r l", l=ln_),
                        scalar1=bih_t[:, m:m + 1])

        # --- recurrence (interleaved directions) ---
        NGC = MC // 4                      # hidden chunks (1 or 2)
        gsb = [rec.tile([128, MC, NB], BF16, name=f"gs{d}")
               for d in range(ND)]
        c_t = [rec.tile([128, NGC, NB], F32, name=f"ct{d}")
               for d in range(ND)]
        ig_t = [rec.tile([128, NGC, NB], BF16, name=f"ig{d}")
               for d in range(ND)]
        tc_t = [rec.tile([128, NGC, NB], BF16, name=f"tc{d}")
               for d in range(ND)]
        slot = 64 if NB <= 64 else 256

        def step(d, l, first):
            lp = l + 1 if (intra and d == 1) else l - 1
            ps = psumS.tile([128, MC, slot], F32, tag=f"lps{d}")
            if intra:
                for m in range(4):
                    st = Qp
                    for k in range(4):
                        o0 = 4 * l + k
                        rhs = bass.AP(
                            tensor=z_cm.tensor, offset=z_cm.offset + o0,
                            ap=[z_cm.ap[0], [st, NB]])
                        nc.tensor.matmul(ps[:, m, :NB],
                                         wih_t[:, d, m, k, :], rhs,
                                         start=(k == 0),
                                         stop=(k == 3 and first))
                    if not first:
                        nc.tensor.matmul(ps[:, m, :NB], whh_t[:, d, m, :],
                                         hbufs[d][:, lp, :],
                                         start=False, stop=True)
            else:
                if not first:
                    for m in range(8):
                        for kc in range(2):
                            nc.tensor.matmul(ps[:, m, :NB],
                                             whh_t[:, m * 2 + kc, :],
                                             hb2[:, kc, lp, :],
                                             start=(kc == 0), stop=(kc == 1))
                    nc.vector.tensor_tensor(out=gsb[d][:], in0=ps[:, :, :NB],
                                            in1=pre2[:, :, :, l], op=OP.add)
            # gates: i (NGC), f (NGC), o (NGC), g (NGC)
            gg = gsb[d]
            if intra:
                sig_in = ps[:, 0:3 * NGC, :NB]
                tanh_in = ps[:, 3 * NGC:, :NB]
            elif first:
                sig_in = pre2[:, 0:3 * NGC, :, l]
                tanh_in = pre2[:, 3 * NGC:, :, l]
            else:
                sig_in = gg[:, 0:3 * NGC, :]
                tanh_in = gg[:, 3 * NGC:, :]
            nc.scalar.activation(out=gg[:, 0:3 * NGC, :], in_=sig_in,
                                 func=AF.Sigmoid)
            nc.scalar.activation(out=gg[:, 3 * NGC:, :], in_=tanh_in,
                                 func=AF.Tanh)
            i_g, f_g = gg[:, 0:NGC, :], gg[:, NGC:2 * NGC, :]
            o_g, g_g = gg[:, 2 * NGC:3 * NGC, :], gg[:, 3 * NGC:, :]
            if first:
                nc.vector.tensor_tensor(out=c_t[d][:], in0=i_g, in1=g_g,
                                        op=OP.mult)
            else:
                nc.vector.tensor_tensor(out=ig_t[d][:], in0=i_g, in1=g_g,
                                        op=OP.mult)
                nc.vector.tensor_tensor(out=c_t[d][:], in0=f_g, in1=c_t[d][:],
                                        op=OP.mult)
                nc.vector.tensor_tensor(out=c_t[d][:], in0=c_t[d][:],
                                        in1=ig_t[d][:], op=OP.add)
            nc.scalar.activation(out=tc_t[d][:], in_=c_t[d][:], func=AF.Tanh)
            if intra:
                nc.vector.tensor_tensor(out=hbufs[d][:, l, :], in0=o_g,
                                        in1=tc_t[d][:], op=OP.mult)
            else:
                nc.vector.tensor_tensor(out=hb2[:, :, l, :], in0=o_g,
                                        in1=tc_t[d][:], op=OP.mult)

        for i in range(L):
            step(0, i, i == 0)
            if intra:
                step(1, L - 1 - i, i == 0)

        # --- ConvT + bias + residual, l-chunked ---
        xu_t = rec.tile([128, 2, NT], F32)
        nc.sync.dma_start(out=xu_t[:], in_=x_u[:])
        CL = 2 if intra else 30
        for mo in range(2):
            for l0 in range(0, L, CL):
                ln_ = min(CL, L - l0)
                nn_ = ln_ * NB
                ps2 = psumB.tile([128, 512], F32, tag="pct")
                nch = 0
                for d in range(ND):
                    for k in range(KC):
                        hsl = (hbufs[d][:, l0:l0 + ln_, :] if intra
                               else hb2[:, k, l0:l0 + ln_, :])
                        nc.tensor.matmul(
                            ps2[:, :nn_], ct_tl[:, d, mo, k, :],
                            hsl.rearrange("p l t -> p (l t)"),
                            start=(nch == 0), stop=(nch == ND * KC - 1))
                        nch += 1
                nc.vector.scalar_tensor_tensor(
                    out=ou[:, mo, l0:l0 + ln_, :].rearrange(
                        "p l t -> p (l t)"),
                    in0=ps2[:, :nn_], scalar=ctb_t[:, mo:mo + 1],
                    in1=xu_t[:, mo, l0 * NB:l0 * NB + nn_],
                    op0=OP.add, op1=OP.add)
        nc.sync.dma_start(out=outu[:],
                          in_=ou[:].rearrange("p a l t -> p a (l t)"))
    nc.compile()
    return nc


# ---------------- Launch 3a: QKV conv + PReLU + LN ----------------

def build_l3a():
    nc = new_nc()
    icm = nc.dram_tensor("icm", [64, TSH, Qp], BF16, kind="ExternalInput")
    wall = nc.dram_tensor("wall", [64, 96], BF16, kind="ExternalInput")
    bs = nc.dram_tensor("bs", [96, 4], F32, kind="ExternalInput")
    # bs cols: bias, alpha, cnt_inv, gscale (per row)
    gmat = nc.dram_tensor("gmat", [96, 96], BF16, kind="ExternalInput")
    qkvo = nc.dram_tensor("qkvo", [96, TSH, Qp], BF16, kind="ExternalOutput")
    NTF = TSH * Qp  # 17000
    ctx = contextlib.ExitStack()
    with tile.TileContext(nc) as tc, ctx:
        const = ctx.enter_context(tc.tile_pool(name="const", bufs=1))
        big = ctx.enter_context(tc.tile_pool(name="big", bufs=1))
        work = ctx.enter_context(tc.tile_pool(name="work", bufs=2))
        psum = ctx.enter_context(tc.tile_pool(name="psum", bufs=2, space="PSUM"))
        eps_t = const.tile([96, 1], F32)
        nc.vector.memset(eps_t[:], EPS)
        ict = big.tile([64, NTF], BF16, tag="ict")
        nc.sync.dma_start(out=ict[:], in_=icm.rearrange("c t f -> c (t f)"))
        wt = const.tile([64, 96], BF16)
        nc.sync.dma_start(out=wt[:], in_=wall[:])
        bst = const.tile([96, 4], F32)
        nc.sync.dma_start(out=bst[:], in_=bs[:])
        gm = const.tile([96, 96], BF16)
        nc.sync.dma_start(out=gm[:], in_=gmat[:])

        qr = big.tile([96, NTF], F32, tag="qr")
        for n0 in range(0, NTF, 512):
            nn_ = min(512, NTF - n0)
            ps = psum.tile([96, 512], F32, tag="pc")
            nc.tensor.matmul(ps[:, :nn_], wt[:], ict[:, n0:n0 + nn_],
                             start=True, stop=True)
            nc.scalar.activation(out=qr[:, n0:n0 + nn_], in_=ps[:, :nn_],
                                 func=AF.Prelu, bias=bst[:, 0:1],
                                 alpha=bst[:, 1:2])
        # stats over (e,f) groups: reduce f, then group-collapse via gmat
        s1 = work.tile([96, TSH], F32, tag="s1")
        nc.vector.tensor_reduce(out=s1[:], in_=qr[:].rearrange(
            "p (t f) -> p t f", f=Qp), axis=AX.X, op=OP.add)
        sq = big.tile([96, NTF], BF16, tag="sq")
        nc.scalar.activation(out=sq[:], in_=qr[:], func=AF.Square)
        s2 = work.tile([96, TSH], F32, tag="s2")
        nc.vector.tensor_reduce(out=s2[:], in_=sq[:].rearrange(
            "p (t f) -> p t f", f=Qp), axis=AX.X, op=OP.add)
        s1b = work.tile([96, TSH], BF16, tag="s1b")
        nc.vector.tensor_copy(out=s1b[:], in_=s1[:])
        s2b = work.tile([96, TSH], BF16, tag="s2b")
        nc.vector.tensor_copy(out=s2b[:], in_=s2[:])
        mu = work.tile([96, TSH], F32, tag="mu")
        ps1 = psum.tile([96, TSH], F32, tag="pg1")
        nc.tensor.matmul(ps1[:], gm[:], s1b[:], start=True, stop=True)
        nc.vector.tensor_scalar_mul(out=mu[:], in0=ps1[:], scalar1=bst[:, 2:3])
        var = work.tile([96, TSH], F32, tag="var")
        ps2g = psum.tile([96, TSH], F32, tag="pg2")
        nc.tensor.matmul(ps2g[:], gm[:], s2b[:], start=True, stop=True)
        nc.vector.tensor_scalar_mul(out=var[:], in0=ps2g[:], scalar1=bst[:, 2:3])
        mu2 = work.tile([96, TSH], F32, tag="mu2")
        nc.vector.tensor_tensor(out=mu2[:], in0=mu[:], in1=mu[:], op=OP.mult)
        nc.vector.tensor_tensor(out=var[:], in0=var[:], in1=mu2[:],
                                op=OP.subtract)
        rs = work.tile([96, TSH], F32, tag="rs")
        nc.scalar.activation(out=rs[:], in_=var[:], func=AF.Sqrt, bias=eps_t[:])
        nc.vector.reciprocal(out=rs[:], in_=rs[:])
        nc.vector.tensor_scalar_mul(out=rs[:], in0=rs[:], scalar1=bst[:, 3:4])
        zh = big.tile([96, TSH, Qp], BF16, tag="zh")
        qr3 = qr[:].rearrange("p (t f) -> p t f", f=Qp)
        nc.vector.tensor_tensor(out=zh[:], in0=qr3, in1=bap(mu, [Qp]),
                                op=OP.subtract)
        nc.vector.tensor_tensor(out=zh[:], in0=zh[:], in1=bap(rs, [Qp]),
                                op=OP.mult)
        nc.vector.memset(zh[:, :, Q:Qp], 0.0)
        nc.sync.dma_start(out=qkvo[:], in_=zh[:])
    nc.compile()
    return nc


# ---------------- Launch 3b: attention per (h,b) ----------------

def build_l3b():
    nc = new_nc()
    # feature-major Q,K (host-transposed): [384, 1000] rows = 272QT+pad, etc
    qT = nc.dram_tensor("qT", [128, 3, T], BF16, kind="ExternalInput")
    kT = nc.dram_tensor("kT", [128, 3, T], BF16, kind="ExternalInput")
    vm = nc.dram_tensor("vm", [128, 8, Dv * Qp], BF16,
                        kind="ExternalInput")
    msk = nc.dram_tensor("msk", [128, 128], F32, kind="ExternalInput")
    avo = nc.dram_tensor("avo", [128, 8, Dv * Qp], BF16,
                         kind="ExternalOutput")
    DFv = Dv * Qp
    ctx = contextlib.ExitStack()
    with tile.TileContext(nc) as tc, ctx:
        const = ctx.enter_context(tc.tile_pool(name="const", bufs=1))
        big = ctx.enter_context(tc.tile_pool(name="big", bufs=1))
        work = ctx.enter_context(tc.tile_pool(name="work", bufs=3))
        psum = ctx.enter_context(tc.tile_pool(name="psum", bufs=2, space="PSUM"))
        psumB = ctx.enter_context(tc.tile_pool(name="psumB", bufs=1,
                                               space="PSUM"))
        ident = const.tile([128, 128], F32)
        make_identity(nc, ident[:])
        qt_t = big.tile([128, 3, T], BF16, tag="qt")
        nc.sync.dma_start(out=qt_t[:], in_=qT[:])
        kt_t = big.tile([128, 3, T], BF16, tag="kt")
        nc.sync.dma_start(out=kt_t[:], in_=kT[:])
        vm_t = big.tile([128, 8, DFv], BF16, tag="vm")
        nc.sync.dma_start(out=vm_t[:], in_=vm[:])
        msk_t = const.tile([128, 128], F32)
        nc.sync.dma_start(out=msk_t[:], in_=msk[:])

        for tcn in range(8):
            ns = min((tcn + 1) * 128, T)
            tch = min(128, T - tcn * 128)
            sc = big.tile([128, 1024], F32, tag="sc")
            for s0 in range(0, ns, 512):
                nn_ = min(512, ns - s0)
                ps = psum.tile([128, 512], F32, tag="psc")
                for kc in range(3):
                    nc.tensor.matmul(
                        ps[:tch, :nn_],
                        qt_t[:, kc, tcn * 128:tcn * 128 + tch],
                        kt_t[:, kc, s0:s0 + nn_],
                        start=(kc == 0), stop=(kc == 2))
                nc.vector.tensor_copy(out=sc[:tch, s0:s0 + nn_],
                                      in_=ps[:tch, :nn_])
            dw = ns - tcn * 128
            nc.vector.tensor_tensor(out=sc[:tch, tcn * 128:ns],
                                    in0=sc[:tch, tcn * 128:ns],
                                    in1=msk_t[:tch, :dw], op=OP.add)
            mx = work.tile([128, 1], F32, tag="mx")
            nc.vector.tensor_reduce(out=mx[:tch], in_=sc[:tch, :ns], axis=AX.X,
                                    op=OP.max)
            nc.vector.tensor_scalar_mul(out=mx[:tch], in0=mx[:tch],
                                        scalar1=-1.0)
            sme = work.tile([128, 1], F32, tag="sme")
            nc.scalar.activation(out=sc[:tch, :ns], in_=sc[:tch, :ns],
                                 func=AF.Exp, bias=mx[:tch],
                                 accum_out=sme[:tch])
            nc.vector.reciprocal(out=sme[:tch], in_=sme[:tch])
            av = psumB.tile([128, 3, 512], F32, tag="pav")
            for sb in range(tcn + 1):
                scb = min(128, ns - sb * 128)
                pT = psum.tile([128, 128], F32, tag="ptr")
                nc.tensor.transpose(pT[:scb, :tch],
                                    sc[:tch, sb * 128:sb * 128 + scb],
                                    ident[:tch, :tch])
                aT = work.tile([128, 128], BF16, tag="aT")
                nc.scalar.copy(out=aT[:scb, :tch], in_=pT[:scb, :tch])
                for n3 in range(3):
                    nn_ = min(512, DFv - n3 * 512)
                    nc.tensor.matmul(av[:tch, n3, :nn_], aT[:scb, :tch],
                                     vm_t[:scb, sb, n3 * 512:n3 * 512 + nn_],
                                     start=(sb == 0), stop=(sb == tcn))
            avs = big.tile([128, DFv], BF16, tag="avs")
            av2 = bass.AP(tensor=av.tensor, offset=av.offset,
                          ap=[av.ap[0], [1, DFv]])
            nc.vector.tensor_scalar_mul(out=avs[:tch], in0=av2[:tch],
                                        scalar1=sme[:tch])
            nc.sync.dma_start(out=avo[:, tcn, :], in_=avs[:])
    nc.compile()
    return nc


# ---------------- Launch 3c: proj + out-LN + residual ----------------

def build_l3c():
    nc = new_nc()
    avf = nc.dram_tensor("avf", [64, TSH, Qp], BF16, kind="ExternalInput")
    icm = nc.dram_tensor("icm", [64, TSH, Qp], F32, kind="ExternalInput")
    pw = nc.dram_tensor("pw", [64, 64], BF16, kind="ExternalInput")
    pb = nc.dram_tensor("pb", [64, 3], F32, kind="ExternalInput")
    # pb cols: bias, gamma0*? , ... col0 bias, col1 alpha-scalar bcast
    outo = nc.dram_tensor("outo", [64, TSH, Q], F32, kind="ExternalOutput")
    NTF = TSH * Qp
    ctx = contextlib.ExitStack()
    with tile.TileContext(nc) as tc, ctx:
        const = ctx.enter_context(tc.tile_pool(name="const", bufs=1))
        big = ctx.enter_context(tc.tile_pool(name="big", bufs=1))
        work = ctx.enter_context(tc.tile_pool(name="work", bufs=1))
        psum = ctx.enter_context(tc.tile_pool(name="psum", bufs=2, space="PSUM"))
        eps_t = const.tile([128, 1], F32)
        nc.vector.memset(eps_t[:], EPS)
        ones_t = const.tile([64, 128], BF16)
        nc.vector.memset(ones_t[:], 1.0)
        avt = big.tile([64, NTF], BF16, tag="avt")
        nc.sync.dma_start(out=avt[:], in_=avf.rearrange("c t f -> c (t f)"))
        pwt = const.tile([64, 64], BF16)
        nc.sync.dma_start(out=pwt[:], in_=pw[:])
        pbt = const.tile([64, 3], F32)
        nc.sync.dma_start(out=pbt[:], in_=pb[:])

        P = big.tile([64, NTF], F32, tag="P")
        for n0 in range(0, NTF, 512):
            nn_ = min(512, NTF - n0)
            ps = psum.tile([64, 512], F32, tag="pp")
            nc.tensor.matmul(ps[:, :nn_], pwt[:], avt[:, n0:n0 + nn_],
                             start=True, stop=True)
            nc.scalar.activation(out=P[:, n0:n0 + nn_], in_=ps[:, :nn_],
                                 func=AF.Prelu, bias=pbt[:, 0:1],
                                 alpha=pbt[:, 1:2])
        P3 = P[:].rearrange("p (t f) -> p t f", f=Qp)
        nc.vector.memset(P3[:, :, Q:Qp], 0.0)
        s1 = work.tile([64, TSH], F32, tag="s1")
        nc.vector.tensor_reduce(out=s1[:], in_=P3, axis=AX.X, op=OP.add)
        sq = big.tile([64, NTF], BF16, tag="avt")
        nc.scalar.activation(out=sq[:], in_=P[:], func=AF.Square)
        s2 = work.tile([64, TSH], F32, tag="s2")
        nc.vector.tensor_reduce(out=s2[:], in_=sq[:].rearrange(
            "p (t f) -> p t f", f=Qp), axis=AX.X, op=OP.add)
        s1b = work.tile([64, TSH], BF16, tag="s1b")
        nc.vector.tensor_copy(out=s1b[:], in_=s1[:])
        s2b = work.tile([64, TSH], BF16, tag="s2b")
        nc.vector.tensor_copy(out=s2b[:], in_=s2[:])
        NCF = 64 * Q  # 4160
        mu = work.tile([128, TSH], F32, tag="mu")
        psg = psum.tile([128, TSH], F32, tag="pg")
        nc.tensor.matmul(psg[:], ones_t[:], s1b[:], start=True, stop=True)
        nc.vector.tensor_scalar_mul(out=mu[:], in0=psg[:], scalar1=1.0 / NCF)
        var = work.tile([128, TSH], F32, tag="var")
        psg2 = psum.tile([128, TSH], F32, tag="pg2")
        nc.tensor.matmul(psg2[:], ones_t[:], s2b[:], start=True, stop=True)
        nc.vector.tensor_scalar_mul(out=var[:], in0=psg2[:], scalar1=1.0 / NCF)
        mu2 = work.tile([128, TSH], F32, tag="mu2")
        nc.vector.tensor_tensor(out=mu2[:], in0=mu[:], in1=mu[:], op=OP.mult)
        nc.vector.tensor_tensor(out=var[:], in0=var[:], in1=mu2[:],
                                op=OP.subtract)
        rs = work.tile([128, TSH], F32, tag="rs")
        nc.scalar.activation(out=rs[:], in_=var[:], func=AF.Sqrt, bias=eps_t[:])
        nc.vector.reciprocal(out=rs[:], in_=rs[:])
        # out = (P - mu)*rs + inter
        o1 = big.tile([64, TSH, Qp], F32, tag="o1")
        nc.vector.tensor_tensor(out=o1[:], in0=P3, in1=bap(mu[0:64, :], [Qp]),
                                op=OP.subtract)
        nc.vector.tensor_tensor(out=o1[:], in0=o1[:], in1=bap(rs[0:64, :], [Qp]),
                                op=OP.mult)
        ict = big.tile([64, NTF], F32, tag="P")
        nc.sync.dma_start(out=ict[:], in_=icm.rearrange("c t f -> c (t f)"))
        nc.vector.tensor_tensor(out=o1[:], in0=o1[:],
                                in1=ict[:].rearrange("p (t f) -> p t f", f=Qp),
                                op=OP.add)
        nc.sync.dma_start(out=outo[:], in_=o1[:, :, :Q])
    nc.compile()
    return nc


# ======================= host side =======================

_CACHE = {}


def _posmajor(arr_pos_c, G):
    """[NPOS, nred] -> [128, G, nred] tiles, pos = g*128+p."""
    npos, nred = arr_pos_c.shape
    pad = np.zeros((G * 128, nred), arr_pos_c.dtype)
    pad[:npos] = arr_pos_c
    return np.ascontiguousarray(pad.reshape(G, 128, nred).transpose(1, 0, 2))


def _lstm_weight_prep(wih, whh, bih, bhh, ctw, ctb, gamma, beta, MC, KC):
    """Fold LN gamma/beta into wih/bias; build device layouts."""
    g = gamma.reshape(-1).astype(np.float64)   # [C]
    b = beta.reshape(-1).astype(np.float64)
    wih = np.asarray(wih, np.float64)          # [4H, C*KS]
    NH4 = wih.shape[0]
    w4 = wih.reshape(NH4, C, KS)
    wih_eff = w4 * g[None, :, None]
    bih_eff = (np.asarray(bih, np.float64) + np.asarray(bhh, np.float64)
               + (w4 * b[None, :, None]).sum((1, 2)))
    # device wih tile [MC, 4, 64, 128]: [m, k, c, gate-in-chunk]
    wt = np.zeros((MC, 4, 64, 128), np.float32)
    for m in range(MC):
        for k in range(4):
            wt[m, k] = wih_eff[m * 128:(m + 1) * 128, :, k].T
    # whh lhsT [MC*KC, 128, 128]: chunk (m,kc): whh[m*128:.., kc*128:..].T
    whh = np.asarray(whh, np.float64)
    wh = np.zeros((MC * KC, 128, 128), np.float32)
    for m in range(MC):
        for kc in range(KC):
            wh[m * KC + kc] = whh[m * 128:(m + 1) * 128,
                                  kc * 128:(kc + 1) * 128].T
    bih_t = np.zeros((128, MC), np.float32)
    for m in range(MC):
        bih_t[:, m] = bih_eff[m * 128:(m + 1) * 128]
    # convT: ctw [HIDd, 64, 4] -> [2, KC*128, 128] ; out rows (k',c) k'*64+c
    ctw = np.asarray(ctw, np.float64)
    KCc = ctw.shape[0] // 128
    ct = np.zeros((2, KCc * 128, 128), np.float32)
    for mo in range(2):
        for kp in range(2):
            for cc in range(64):
                j = kp * 64 + cc
                ct[mo, :, j] = ctw[:, cc, mo * 2 + kp]
    ctb_t = np.zeros((128, 2), np.float32)
    for mo in range(2):
        for kp in range(2):
            ctb_t[kp * 64:(kp + 1) * 64, mo] = np.asarray(ctb)
    return wt, wh, bih_t, ct, ctb_t


def _unf_rows(arr_c_t, L, off=0):
    """arr [64, NTIME] -> x_u [2, 128, L, NB] rows (k,c) k*64+c, cols (l, nb).
    value = arr[c, nb, 4l+k+off] where arr is [64, NB, NTIME-per-row]."""
    C_, NB, NT_ = arr_c_t.shape
    out = np.zeros((2, 128, L, NB), np.float32)
    for mo in range(2):
        for kp in range(2):
            k = mo * 2 + kp
            idx = 4 * np.arange(L) + k + off
            v = arr_c_t[:, :, :][:, :, idx]          # [64, NB, L]
            out[mo, kp * 64:(kp + 1) * 64] = v.transpose(0, 2, 1)
    return out


def _uniform(a):
    a = np.asarray(a)
    assert np.all(a == a.flat[0]), "nonuniform LN affine not supported"
    return float(a.flat[0])


def _prep_lstm_v2(wih, whh, bih, bhh, gamma, beta):
    """LN-folded, gate-reordered (i,f,o,g) weight arrays."""
    g = np.asarray(gamma, np.float64).reshape(-1)
    b = np.asarray(beta, np.float64).reshape(-1)
    NH4 = np.asarray(wih).shape[0]
    w4 = np.asarray(wih, np.float64).reshape(NH4, C, KS)
    wih_eff = w4 * g[None, :, None]
    bih_eff = (np.asarray(bih, np.float64) + np.asarray(bhh, np.float64)
               + (w4 * b[None, :, None]).sum((1, 2)))
    H = NH4 // 4
    perm = np.r_[0:H, H:2 * H, 3 * H:4 * H, 2 * H:3 * H]
    return wih_eff[perm], bih_eff[perm], np.asarray(whh, np.float64)[perm]


def kernel(**inputs):
    ii = {k: np.asarray(v) for k, v in inputs.items()}
    x = ii["x"].astype(np.float32)
    xp = np.zeros((B, C, T, Qp), np.float32)
    xp[:, :, :, :Q] = x

    if "l1" not in _CACHE:
        _CACHE["l1"] = build_lstm_launch("intra")
        _CACHE["l2"] = build_lstm_launch("inter")
        _CACHE["l3a"] = build_l3a()
        _CACHE["l3b"] = build_l3b()
        _CACHE["l3c"] = build_l3c()

    bf = lambda a: np.ascontiguousarray(a, dtype=np.float32).astype(
        mybir.dt.np(BF16))
    f32c = lambda a: np.ascontiguousarray(a, dtype=np.float32)

    # ---------- L1 ----------
    wts, whs = [], []
    for d in range(2):
        we, be, wp = _prep_lstm_v2(
            ii["intra_wih"][d], ii["intra_whh"][d], ii["intra_bih"][d],
            ii["intra_bhh"][d], ii["intra_gamma"], ii["intra_beta"])
        wt = np.zeros((65, 4, 4, 128), np.float32)
        wh = np.zeros((128, 4, 128), np.float32)
        for m in range(4):
            for k in range(4):
                wt[:64, m, k] = we[m * 128:(m + 1) * 128, :, k].T
            wt[64, m, 0] = be[m * 128:(m + 1) * 128]
            wh[:, m] = wp[m * 128:(m + 1) * 128].T
        wts.append(wt); whs.append(wh)
    _, _, _, ct1, ctb1 = _lstm_weight_prep(
        ii["intra_wih"][0], ii["intra_whh"][0], ii["intra_bih"][0],
        ii["intra_bhh"][0], ii["intra_ct_w"], ii["intra_ct_b"],
        ii["intra_gamma"], ii["intra_beta"], 4, 1)
    # intra ctw [256,64,4]: split fw rows 0:128, bw 128:256 across d
    ctw_i = np.asarray(ii["intra_ct_w"], np.float64)
    ct_d = np.zeros((2, 2, 128, 128), np.float32)
    for d in range(2):
        sub = ctw_i[d * 128:(d + 1) * 128]
        for mo in range(2):
            for kp in range(2):
                for cc in range(64):
                    ct_d[d, mo, :, kp * 64 + cc] = sub[:, cc, mo * 2 + kp]
    l1_w = {
        "wih": bf(np.stack(wts, axis=1)),
        "whh": bf(np.stack(whs, axis=1)),
        "ctw": bf(ct_d.reshape(2, 2, 1, 128, 128).transpose(3, 0, 1, 2, 4)),
        "ctb": f32c(ctb1),
    }
    l1_maps = []
    for core in range(NCORES):
        b = core // 4
        t0 = (core % 4) * TSH
        xs = xp[b, :, t0:t0 + TSH, :]                    # [C, TSH, Qp]
        x_pm = _posmajor(np.ascontiguousarray(
            xs.transpose(1, 2, 0)).reshape(NP1, C), G1)
        xu = _unf_rows(xs.transpose(0, 1, 2).reshape(C, TSH, Qp)
                       .transpose(0, 1, 2), L1)          # wait: per row=t
        # arr [64, NB=TSH, Qp]
        xu = _unf_rows(np.ascontiguousarray(xs.transpose(0, 1, 2)), L1)
        l1_maps.append({**l1_w, "x_pm": x_pm,
                        "x_u": f32c(xu.reshape(2, 128, L1 * TSH)
                                    .transpose(1, 0, 2))})
    r1 = run_bass_kernel_spmd(_CACHE["l1"], l1_maps,
                              core_ids=list(range(NCORES))).results
    # reassemble intra [B, C, T, Qp]
    intra = np.zeros((B, C, T, Qp), np.float32)
    for core in range(NCORES):
        b = core // 4
        t0 = (core % 4) * TSH
        ou = r1[core]["outu"].transpose(1, 0, 2).reshape(2, 128, L1, TSH)
        for mo in range(2):
            for kp in range(2):
                k = mo * 2 + kp
                q_idx = 4 * np.arange(L1) + k
                intra[b, :, t0:t0 + TSH, q_idx] = \
                    ou[mo, kp * 64:(kp + 1) * 64].transpose(1, 0, 2)
    # ---------- L2 ----------
    _, _, _, ct2, ctb2 = _lstm_weight_prep(
        ii["inter_wih"], ii["inter_whh"], ii["inter_bih"], ii["inter_bhh"],
        ii["inter_ct_w"], ii["inter_ct_b"], ii["inter_gamma"],
        ii["inter_beta"], 8, 2)
    we2, be2, wp2 = _prep_lstm_v2(
        ii["inter_wih"], ii["inter_whh"], ii["inter_bih"], ii["inter_bhh"],
        ii["inter_gamma"], ii["inter_beta"])
    wih2 = np.zeros((128, 8, 2, 128), np.float32)
    whh2 = np.zeros((128, 16, 128), np.float32)
    bih2 = np.zeros((128, 8), np.float32)
    for m in range(8):
        rows = we2[m * 128:(m + 1) * 128]
        for kp in range(2):
            wih2[:64, m, kp] = rows[:, :, 2 * kp].T
            wih2[64:, m, kp] = rows[:, :, 2 * kp + 1].T
        for kc in range(2):
            whh2[:, m * 2 + kc] = wp2[m * 128:(m + 1) * 128,
                                      kc * 128:(kc + 1) * 128].T
        bih2[:, m] = be2[m * 128:(m + 1) * 128]
    l2_w = {"wih": bf(wih2), "whh": bf(whh2), "bih": f32c(bih2),
            "ctw": bf(ct2.reshape(2, 2, 128, 128).transpose(2, 0, 1, 3)
                      .reshape(128, 1, 2, 2, 128)),
            "ctb": f32c(ctb2)}
    l2_maps = []
    for core in range(NCORES):
        b = core // 4
        q0 = (core % 4) * RW2
        isl = intra[b, :, :, q0:q0 + RW2]                # [C, T, RW2]
        rows_ct = np.ascontiguousarray(isl.transpose(0, 2, 1))  # [C,RW2,T]
        x_pm = _posmajor(np.ascontiguousarray(
            rows_ct.transpose(1, 2, 0)).reshape(RW2 * T, C), G1)
        # x_u resid: value = intra[c, row, t=4l+k]
        xu = _unf_rows(rows_ct, L2, off=0)
        l2_maps.append({**l2_w, "x_pm": x_pm,
                        "x_u": f32c(xu.reshape(2, 128, L2 * RW2)
                                    .transpose(1, 0, 2))})
    r2 = run_bass_kernel_spmd(_CACHE["l2"], l2_maps,
                              core_ids=list(range(NCORES))).results
    inter = np.zeros((B, C, T, Qp), np.float32)
    for core in range(NCORES):
        b = core // 4
        q0 = (core % 4) * RW2
        ou = r2[core]["outu"].transpose(1, 0, 2).reshape(2, 128, L2, RW2)
        for mo in range(2):
            for kp in range(2):
                k = mo * 2 + kp
                t_idx = 4 * np.arange(L2) + k
                inter[b, :, t_idx, q0:q0 + RW2] = \
                    ou[mo, kp * 64:(kp + 1) * 64].transpose(1, 0, 2)
    inter_r = np.zeros((B, C, T, Qp), np.float32)
    inter_r[:, :, :, :Q] = inter[:, :, :, :Q]            # real freqs only
    # ---------- L3a ----------
    qg = _uniform(ii["q_g"]); kg = _uniform(ii["k_g"]); vg = _uniform(ii["v_g"])
    assert _uniform(ii["q_bt"]) == 0 and _uniform(ii["k_bt"]) == 0
    assert _uniform(ii["v_bt"]) == 0
    wall = np.zeros((64, 96), np.float32)
    bias96 = np.zeros((96,), np.float32)
    alpha96 = np.zeros((96,), np.float32)
    cnt96 = np.zeros((96,), np.float32)
    gs96 = np.zeros((96,), np.float32)
    grp = np.zeros((96,), np.int32)
    for h in range(NH):
        wall[:, h * 4:h * 4 + 4] = np.asarray(ii["q_w"][h]).T
        wall[:, 16 + h * 4:16 + h * 4 + 4] = np.asarray(ii["k_w"][h]).T
        wall[:, 32 + h * 16:32 + h * 16 + 16] = np.asarray(ii["v_w"][h]).T
        bias96[h * 4:h * 4 + 4] = np.asarray(ii["q_b"][h])
        bias96[16 + h * 4:16 + h * 4 + 4] = np.asarray(ii["k_b"][h])
        alpha96[h * 4:h * 4 + 4] = float(ii["q_p"][h])
        alpha96[16 + h * 4:16 + h * 4 + 4] = float(ii["k_p"][h])
        alpha96[32 + h * 16:32 + h * 16 + 16] = float(ii["v_p"][h])
        cnt96[h * 4:h * 4 + 4] = 1.0 / (E * Q)
        cnt96[16 + h * 4:16 + h * 4 + 4] = 1.0 / (E * Q)
        cnt96[32 + h * 16:32 + h * 16 + 16] = 1.0 / (Dv * Q)
        gs96[h * 4:h * 4 + 4] = qg / np.sqrt(E * Q)
        gs96[16 + h * 4:16 + h * 4 + 4] = kg
        gs96[32 + h * 16:32 + h * 16 + 16] = vg
        grp[h * 4:h * 4 + 4] = h
        grp[16 + h * 4:16 + h * 4 + 4] = 4 + h
        grp[32 + h * 16:32 + h * 16 + 16] = 8 + h
    gmat = (grp[:, None] == grp[None, :]).astype(np.float32)
    bs96 = np.stack([bias96, alpha96, cnt96, gs96], axis=1)
    l3a_w = {"wall": bf(wall), "bs": f32c(bs96), "gmat": bf(gmat)}
    l3a_maps = []
    for core in range(NCORES):
        b = core // 4
        t0 = (core % 4) * TSH
        l3a_maps.append({**l3a_w,
                         "icm": bf(inter_r[b, :, t0:t0 + TSH, :])})
    r3a = run_bass_kernel_spmd(_CACHE["l3a"], l3a_maps,
                               core_ids=list(range(NCORES))).results
    qkv = np.zeros((B, 96, T, Qp), np.float32)
    for core in range(NCORES):
        b = core // 4
        t0 = (core % 4) * TSH
        qkv[b, :, t0:t0 + TSH, :] = r3a[core]["qkvo"].astype(
            np.float32).transpose(0, 1, 2)
    # ---------- L3b ----------
    mask = np.triu(np.full((128, 128), -1e9, np.float32), 1)
    l3b_maps = []
    for core in range(NCORES):
        h, b = core % 4, core // 4
        qh = qkv[b, h * 4:h * 4 + 4]                     # [4, T, Qp]
        kh = qkv[b, 16 + h * 4:16 + h * 4 + 4]
        vh = qkv[b, 32 + h * 16:32 + h * 16 + 16]        # [16, T, Qp]
        qT = np.zeros((384, T), np.float32)
        kT = np.zeros((384, T), np.float32)
        qT[:4 * Qp] = qh.transpose(0, 2, 1).reshape(4 * Qp, T)
        kT[:4 * Qp] = kh.transpose(0, 2, 1).reshape(4 * Qp, T)
        vm = np.zeros((8, 128, Dv * Qp), np.float32)
        vflat = vh.transpose(1, 0, 2).reshape(T, Dv * Qp)  # [s, (d,f)]
        vm.reshape(1024, Dv * Qp)[:T] = vflat
        l3b_maps.append({"qT": bf(qT.reshape(3, 128, T).transpose(1, 0, 2)),
                         "kT": bf(kT.reshape(3, 128, T).transpose(1, 0, 2)),
                         "vm": bf(vm.transpose(1, 0, 2)), "msk": f32c(mask)})
    r3b = run_bass_kernel_spmd(_CACHE["l3b"], l3b_maps,
                               core_ids=list(range(NCORES))).results
    # av: [B, (h,d) 64, T, Qp]
    av = np.zeros((B, 64, T, Qp), np.float32)
    for core in range(NCORES):
        h, b = core % 4, core // 4
        a_ = r3b[core]["avo"].astype(np.float32).transpose(1, 0, 2)\
            .reshape(1024, Dv, Qp)[:T]
        av[b, h * 16:(h + 1) * 16] = a_.transpose(1, 0, 2)
    # ---------- L3c ----------
    assert _uniform(ii["proj_g"]) == 1.0 and _uniform(ii["proj_bt"]) == 0.0
    pw = np.asarray(ii["proj_w"], np.float32).T          # lhsT [hd, c]
    pb3 = np.zeros((64, 3), np.float32)
    pb3[:, 0] = np.asarray(ii["proj_b"])
    pb3[:, 1] = float(ii["proj_p"])
    l3c_w = {"pw": bf(pw), "pb": f32c(pb3)}
    l3c_maps = []
    for core in range(NCORES):
        b = core // 4
        t0 = (core % 4) * TSH
        l3c_maps.append({**l3c_w, "avf": bf(av[b, :, t0:t0 + TSH, :]),
                         "icm": f32c(inter_r[b, :, t0:t0 + TSH, :])})
    r3c = run_bass_kernel_spmd(_CACHE["l3c"], l3c_maps,
                               core_ids=list(range(NCORES))).results
    out = np.zeros((B, C, T, Q), np.float32)
    for core in range(NCORES):
        b = core // 4
        t0 = (core % 4) * TSH
        out[b, :, t0:t0 + TSH, :] = r3c[core]["outo"]
    kernel.dbg = {"intra": intra, "inter": inter, "qkv": qkv, "av": av}
    return out

